# revision 45
# baseline (speedup 1.0000x reference)
"""DyRep classifier Bass kernel for 8 Trainium2 NeuronCores.

Strategy (self-contained; shapes hardcoded for the target problem):
  - The output depends only on per-label-node rows of (memory_buf,
    node_state[post-event], last_seen[post-event], node_features).
  - Host packs the four per-node tables into one 768-byte row per node
    (vs 1024B for the bf16-only layout):
      block0 (256B): memory_buf bf16 — DVE operand.
      block1 (256B): node_state bf16 — GRU matmul rhs / DVE operand.
      block2 (256B): dec_pre=exp(relu(decay)*last_seen) bf16 at u16-unit
        0, feat*4 as fp8 pairs on units 1..127 (even bytes feat[1:128],
        odd bytes feat[128:172] on units 1..44 and feat[0] on unit 45).
    dec_pre is in [1,1.65) so BOTH its bytes decode as finite fp8 — the
    feat matmuls span it with zero weight rows without NaN risk. The
    feat matmuls run fp8 x fp8 (weights also *4; HW runs mixed
    bf16 x fp8 matmuls incorrectly, and fp8 weights at natural scale
    would be subnormal). The combined x16 is undone for free by the
    ACT scale on the h1 relu; W1 is pre-multiplied by 16 to match.
    No exp on device: dec = dec_pre * exp(-rd*T) via an ACT scale on a
    [1,S] slice, broadcast across partitions by a K=1 ones-matmul.
    Sharded row-wise into 16 chunks of 31250 rows (int16-indexable);
    each of the 8 cores owns 2 chunks.
  - Host routes each unique label node to its owner (core, chunk) and
    splits into "untouched" / "touched" (touched = node hit by the
    event batch, needs the GRU update applied on the fly).
  - Device per core: dma_gather(transpose=True) delivers rows
    feature-major. h1 psum accumulates 3 matmuls (featA fp8, featB fp8,
    W1@(mem + state*dec)), ACT relu with fused bias+scale, then W2
    (zero-padded to 64 rows) col-tiled so both supertiles of a gather
    land in ONE [128, 448] psum bank (s0 -> partitions 0:64,
    s1 -> 64:128), one DVE bias-copy, bf16 half-width output.
  - Host unpermutes the per-core outputs back to label order.
"""

import functools
import os
import numpy as np
import ml_dtypes

import concourse.bass as bass
import concourse.mybir as mybir
import concourse.tile as tile
from concourse import bacc
from concourse.bass_utils import run_bass_kernel_spmd

BF16 = ml_dtypes.bfloat16
FP8 = ml_dtypes.float8_e4m3

# Problem dims (fixed by the task)
N = 500000
H = 128
F = 172
C = 50
B = 200000

NCORES = 8
NCHUNK = 16                  # index chunks (int16 addressing limit)
CH = N // NCHUNK             # 31250 rows per chunk
ROW = 512                    # bf16-unit elements per packed row (1024 bytes)
G = int(os.environ.get("K_G", "896"))   # occurrences per gather
# (transpose dma_gather num_idxs is HW-capped at 1016: 896 largest %128)
S = G // 2                   # supertile: occurrences per compute slice
FSC = 4.0                    # fp8 scale for feat AND its weights (x16 total)
OC = 114                     # packed output partitions (2 x C, col-tiled)
NSWQ = int(os.environ.get("K_NSWQ", "4"))  # SWDGE queues: alternate gathers
# across queues so one gather's ring drain overlaps the next gather's
# descriptor generation (each queue has its own 256-desc ring).

f32 = mybir.dt.float32
bf16 = mybir.dt.bfloat16
fp8 = mybir.dt.float8e4
i16 = mybir.dt.int16
AF = mybir.ActivationFunctionType
OP = mybir.AluOpType


def _wrap_idxs(idx: np.ndarray) -> np.ndarray:
    """Wrap a flat int16 index list into the [128, n/16] SWDGE layout:
    element j at [j%16, j//16], replicated into all 8 16-partition groups."""
    n = idx.shape[0]
    assert n % 16 == 0
    cols = n // 16
    t = np.empty((128, cols), dtype=np.int16)
    blk = idx.reshape(cols, 16).T  # [16, cols]
    for k in range(8):
        t[k * 16:(k + 1) * 16, :] = blk
    return t


def build_program(u_pad: int, t_pad: int, ch: int = CH):
    """Build + compile the SPMD Bass program. Cached by padded sizes."""
    nc = bacc.Bacc("TRN2", target_bir_lowering=False, debug=False,
                   num_devices=NCORES, num_swdge_queues=NSWQ)

    dt_in = {}

    def din(name, shape, dt):
        dt_in[name] = nc.dram_tensor(name, shape, dt, kind="ExternalInput").ap()
        return dt_in[name]

    tab_a = din("tab_a", (ch, ROW), bf16)
    tab_b = din("tab_b", (ch, ROW), bf16)
    # Packed weight tensors: 3 DMAs instead of ~20 serialized ones.
    # fwi: [uidx_a | uidx_b | tidx_a | tidx_b] int16
    iu, it = u_pad // 16, t_pad // 16
    fwi = din("fwi", (128, 2 * iu + 2 * it), i16)
    # fwb: [w1t | w2t | whhrt | whhzt | whhnt] bf16 + [wfab bitcast] handled
    # separately (fp8): wfab as DoubleRow planes: [k,0,:] even-byte weights
    # (feat[k], row0 zero), [k,1,:] odd-byte weights (feat[127+k] for
    # k=1..44, feat[0] at k=45, zero elsewhere); all *4
    wfab = din("wfab", (128, 2, 128), fp8)
    fwb = din("fwb", (128, 576), bf16)
    # fwf: [b1p | b2v2 | c_r | c_z | gin | bhn | dect | dtc] f32
    fwf = din("fwf", (128, 8), f32)

    tch = (u_pad + t_pad)                    # out columns per core (half)
    out = nc.dram_tensor("out", (OC, tch), bf16, kind="ExternalOutput").ap()

    class W:
        pass

    with tile.TileContext(nc) as tc:
        with tc.tile_pool(name="wpool", bufs=1) as wp:
            fwi_t = wp.tile([128, 2 * iu + 2 * it], i16, tag="fwi")
            nc.sync.dma_start(fwi_t[:], fwi[:])
            wfab_t = wp.tile([128, 2, 128], fp8, tag="wfab")
            nc.scalar.dma_start(wfab_t[:], wfab[:])
            fwb_t = wp.tile([128, 576], bf16, tag="fwb")
            nc.sync.dma_start(fwb_t[:], fwb[:])
            fwf_t = wp.tile([128, 8], f32, tag="fwf")
            nc.scalar.dma_start(fwf_t[:], fwf[:])
            W.uidx_a = fwi_t[:, 0:iu]
            W.uidx_b = fwi_t[:, iu:2 * iu]
            W.tidx_a = fwi_t[:, 2 * iu:2 * iu + it]
            W.tidx_b = fwi_t[:, 2 * iu + it:2 * iu + 2 * it]
            W.wfab = wfab_t
            W.w1t = fwb_t[:, 0:128]
            W.w2t = fwb_t[:, 128:192]
            W.whhrt = fwb_t[:, 192:320]
            W.whhzt = fwb_t[:, 320:448]
            W.whhnt = fwb_t[:, 448:576]
            W.b1p = fwf_t[:, 0:1]
            W.b2v2 = fwf_t[0:OC, 1:2]
            W.c_r = fwf_t[:, 2:3]
            W.c_z = fwf_t[:, 3:4]
            W.gin = fwf_t[:, 4:5]
            W.bhn = fwf_t[:, 5:6]
            W.dect = fwf_t[:, 6:7]
            W.dtc = fwf_t[:, 7:8]

            def fab_mm(p_h1, X8, s):
                """Open the h1 psum group: featA+featB in ONE DoubleRow
                matmul consuming the raw fp8 byte pairs of block2."""
                rhs = X8[:, 2, bass.ds(2 * s * S, 2 * S)].rearrange(
                    "p (n two) -> p two n", two=2)
                nc.tensor.matmul(p_h1[:], lhsT=W.wfab[:], rhs=rhs,
                                 start=True, stop=False,
                                 perf_mode=mybir.MatmulPerfMode.DoubleRow)

            def finish_tile(sb, ps2, p_o, X, s, p_h1, sstate):
                """Close the h1 group (W1 matmul), relu, W2 into the
                col-tiled p_o half."""
                sl = bass.ds(s * S, S)
                memT = X[:, 0, sl]
                t1 = sb.tile([128, S], bf16, tag="t1", bufs=4)
                nc.vector.tensor_tensor(out=t1[:], in0=sstate[:], in1=memT,
                                        op=OP.add)
                nc.tensor.matmul(p_h1[:], lhsT=W.w1t, rhs=t1[:],
                                 start=False, stop=True)
                h1 = sb.tile([128, S], bf16, tag="h1s", bufs=4)
                nc.scalar.activation(h1[:], p_h1[:], AF.Relu, bias=W.b1p,
                                     scale=1.0 / 16.0)
                o_sl = p_o[0:64, :] if s == 0 else p_o[64:128, :]
                nc.tensor.matmul(o_sl, lhsT=W.w2t, rhs=h1[:],
                                 start=True, stop=True)

            def untouched_gather(sb, ps, ps2, p_o, X):
                """Both supertiles of one gather, phase-split so the
                gather-ready work (feat DoubleRow matmuls, decayed-state
                scaling off the gather-broadcast dec block) issues before
                the dependent W1/W2 chain."""
                X8 = X[:].bitcast(fp8)
                p_h1, sstates = [], []
                for s in range(2):
                    ph = ps2.tile([128, S], f32, tag="h1")
                    fab_mm(ph, X8, s)
                    p_h1.append(ph)
                    sl = bass.ds(s * S, S)
                    sstate = sb.tile([128, S], bf16, tag="sstate", bufs=4)
                    nc.vector.scalar_tensor_tensor(
                        out=sstate[:], in0=X[:, 1, sl], scalar=W.dtc,
                        in1=X[:, 3, sl], op0=OP.mult, op1=OP.mult)
                    sstates.append(sstate)
                for s in range(2):
                    finish_tile(sb, ps2, p_o, X, s, p_h1[s], sstates[s])

            def touched_supertile(sb, ps, ps2, p_o, X, s):
                sl = bass.ds(s * S, S)
                X8 = X[:].bitcast(fp8)
                stT = X[:, 1, sl]
                if True:
                    p_r = ps.tile([128, S], f32, tag="gr")
                    nc.tensor.matmul(p_r[:], lhsT=W.whhrt, rhs=stT,
                                     start=True, stop=True)
                    p_z = ps.tile([128, S], f32, tag="gz")
                    nc.tensor.matmul(p_z[:], lhsT=W.whhzt, rhs=stT,
                                     start=True, stop=True)
                    p_n = ps.tile([128, S], f32, tag="gn")
                    nc.tensor.matmul(p_n[:], lhsT=W.whhnt, rhs=stT,
                                     start=True, stop=True)
                    r = sb.tile([128, S], f32, tag="r")
                    nc.scalar.activation(r[:], p_r[:], AF.Sigmoid, bias=W.c_r)
                    z = sb.tile([128, S], f32, tag="z")
                    nc.scalar.activation(z[:], p_z[:], AF.Sigmoid, bias=W.c_z)
                    hn = sb.tile([128, S], f32, tag="hn")
                    nc.scalar.activation(hn[:], p_n[:], AF.Identity, bias=W.bhn)
                    rn = sb.tile([128, S], f32, tag="rn")
                    nc.vector.tensor_tensor(out=rn[:], in0=r[:], in1=hn[:],
                                            op=OP.mult)
                    n = sb.tile([128, S], f32, tag="n")
                    nc.scalar.activation(n[:], rn[:], AF.Tanh, bias=W.gin)
                    d = sb.tile([128, S], f32, tag="d")
                    nc.vector.tensor_tensor(out=d[:], in0=stT, in1=n[:],
                                            op=OP.subtract)
                    zd = sb.tile([128, S], f32, tag="zd")
                    nc.vector.tensor_tensor(out=zd[:], in0=z[:], in1=d[:],
                                            op=OP.mult)
                    ns = sb.tile([128, S], f32, tag="ns")
                    nc.vector.tensor_tensor(out=ns[:], in0=n[:], in1=zd[:],
                                            op=OP.add)
                    sstate = sb.tile([128, S], bf16, tag="sstate", bufs=4)
                    nc.vector.tensor_scalar_mul(sstate[:], ns[:],
                                                W.dect)

                p_h1 = ps2.tile([128, S], f32, tag="h1")
                fab_mm(p_h1, X8, s)
                finish_tile(sb, ps2, p_o, X, s, p_h1, sstate)

            gctr = [0]

            def stream(gp, sb, ps, ps2, pso, table_ap, idx_tile, n_occ,
                       col0h, touched):
                """Process one (chunk, touched?) stream of n_occ occurrences
                in gathers of G; outputs to out[:, col0h : col0h+n_occ/2]."""
                n_g = n_occ // G
                for g in range(n_g):
                    X = gp.tile([128, 4, G], bf16, tag="gath")
                    nc.gpsimd.dma_gather(
                        out_ap=X[:],
                        in_ap=table_ap[:],
                        idxs_ap=idx_tile[:, bass.ds(g * G // 16, G // 16)],
                        num_idxs=G,
                        num_idxs_reg=G,
                        elem_size=ROW,
                        transpose=True,
                        queue_num=gctr[0] % NSWQ,
                    )
                    gctr[0] += 1
                    p_o = pso.tile([128, S], f32, tag="po")
                    if touched:
                        for s in range(2):
                            touched_supertile(sb, ps, ps2, p_o, X, s)
                    else:
                        untouched_gather(sb, ps, ps2, p_o, X)
                    osb = gp.tile([OC, S], bf16, tag="osb")
                    nc.scalar.activation(osb[:], p_o[0:OC, :], AF.Identity,
                                         bias=W.b2v2)
                    nc.sync.dma_start(
                        out[:, bass.ds(col0h + g * S, S)], osb[:])

            # One deep gather pool shared by both phases so the Pool engine
            # streams descriptor generation continuously across the
            # touched->untouched transition. Touched first: its serial GRU
            # chain drains while untouched gathers already issue.
            # Single pool scope for both phases (a pool-close between the
            # touched and untouched streams costs a ~20us all-queue drain).
            with tc.tile_pool(name="gp", bufs=8) as gp, \
                 tc.tile_pool(name="sb", bufs=2) as sb, \
                 tc.tile_pool(name="ps", bufs=1, space="PSUM") as ps, \
                 tc.tile_pool(name="ps2", bufs=3, space="PSUM") as ps2, \
                 tc.tile_pool(name="pso", bufs=2, space="PSUM") as pso:
                stream(gp, sb, ps, ps2, pso, tab_a, W.tidx_a, t_pad,
                       u_pad // 2, True)
                stream(gp, sb, ps, ps2, pso, tab_b, W.tidx_b, t_pad,
                       u_pad + t_pad // 2, True)
                stream(gp, sb, ps, ps2, pso, tab_a, W.uidx_a, u_pad,
                       0, False)
                stream(gp, sb, ps, ps2, pso, tab_b, W.uidx_b, u_pad,
                       (u_pad + t_pad) // 2, False)

    nc.compile()
    return nc


@functools.lru_cache(maxsize=4)
def _cached_program(u_pad, t_pad, ch):
    return build_program(u_pad, t_pad, ch)


def _round_up(x, m):
    return ((x + m - 1) // m) * m


def _prepare(label_nodes, src, dst, t, msg, memory_buf, node_state, last_seen,
             node_features, decay, W_msg, b_msg, W_ih, W_hh, b_ih, b_hh,
             W_feat, b_feat, W1, b1, W2, b2, current_time):
    """Host-side routing/packing. Returns (in_maps, meta)."""
    label_nodes = np.asarray(label_nodes)

    # ---- host: event-level scalars (O(1) work) ----
    t0 = float(np.asarray(t)[0])
    T = float(current_time)
    rdecay = max(float(decay), 0.0)
    event_msg = msg[0].astype(np.float64) @ W_msg.T.astype(np.float64) + b_msg
    gi = event_msg @ W_ih.T.astype(np.float64) + b_ih  # [3H], includes b_ih
    gi = gi.astype(np.float32)
    dec_t = np.float32(np.exp(-rdecay * (T - t0)))

    # ---- host: routing (dedup to unique label nodes) ----
    touched_nodes = np.unique(np.concatenate([src, dst]))
    uniq_vals, inv = np.unique(label_nodes, return_inverse=True)
    is_t = np.isin(uniq_vals, touched_nodes)
    chunk_id = uniq_vals // CH            # 0..15
    local = (uniq_vals % CH).astype(np.int16)

    key = chunk_id.astype(np.int64) * 2 + is_t
    order = np.argsort(key, kind="stable")
    counts = np.bincount(key, minlength=NCHUNK * 2)
    u_counts = counts[0::2]
    t_counts = counts[1::2]
    u_pad = max(_round_up(int(u_counts.max()), G), G)
    t_pad = max(_round_up(int(t_counts.max()), G), G)

    starts = np.zeros(NCHUNK * 2 + 1, dtype=np.int64)
    np.cumsum(counts, out=starts[1:])

    # ---- host: packed 768B-row table ----
    tab8 = np.zeros((N, 2 * ROW), dtype=np.uint8)
    tab8[:, 0:256] = memory_buf.astype(BF16).view(np.uint8)
    tab8[:, 256:512] = node_state.astype(BF16).view(np.uint8)
    blk = np.zeros((N, 256), dtype=np.uint8)
    f8 = (node_features * FSC).astype(FP8).view(np.uint8)
    dec_pre = np.exp(rdecay * last_seen.astype(np.float64)).astype(BF16)
    blk[:, 0:2] = dec_pre.reshape(-1, 1).view(np.uint8)
    blk[:, 2:256:2] = f8[:, 1:128]        # featA: even bytes, units 1..127
    blk[:, 3:91:2] = f8[:, 128:172]       # featB: odd bytes, units 1..44
    blk[:, 91] = f8[:, 0]                 # feat[0]: odd byte, unit 45
    tab8[:, 512:768] = blk
    # block3: dec_pre replicated into all 128 units -> the transpose
    # gather broadcasts it across partitions for free.
    tab8[:, 768:1024] = np.broadcast_to(
        dec_pre.reshape(-1, 1, 1).view(np.uint8), (N, 128, 2)).reshape(N, 256)
    tab = tab8.view(BF16)                 # [N, 512] bf16-typed bytes

    # ---- host: weights / aux ----
    def bfc(x):
        return np.ascontiguousarray(x, dtype=BF16)

    def f32c(x):
        return np.ascontiguousarray(x, dtype=np.float32).reshape(-1, 1)

    WcT = (W1 @ W_feat).T * FSC  # [F, H]; W_feat folded through W1, x4
    wfab = np.zeros((128, 2, H), np.float32)
    wfab[1:128, 0] = WcT[1:128]           # even-byte plane: feat[1:128]
    wfab[1:45, 1] = WcT[128:172]          # odd-byte plane: feat[128:172]
    wfab[45, 1] = WcT[0]                  # feat[0] at odd byte of unit 45
    b2v2 = np.zeros(OC, np.float32)
    b2v2[0:C] = b2
    b2v2[64:64 + C] = b2
    fwb = np.concatenate([
        bfc(W1.T * (FSC * FSC)),
        bfc(np.concatenate(
            [W2.T, np.zeros((H, 64 - C), np.float32)], axis=1)),
        bfc(W_hh[0:128].T), bfc(W_hh[128:256].T), bfc(W_hh[256:384].T),
    ], axis=1)
    fwf = np.zeros((128, 8), np.float32)
    fwf[:, 0] = b1 + W1 @ b_feat
    fwf[0:OC, 1] = b2v2
    fwf[:, 2] = gi[0:128] + b_hh[0:128]
    fwf[:, 3] = gi[128:256] + b_hh[128:256]
    fwf[:, 4] = gi[256:384]
    fwf[:, 5] = b_hh[256:384]
    fwf[:, 6] = dec_t
    fwf[:, 7] = np.exp(-rdecay * T)
    aux = {
        "wfab": np.ascontiguousarray(wfab, dtype=FP8),
        "fwb": np.ascontiguousarray(fwb),
        "fwf": fwf,
    }

    # ---- host: per-core input maps ----
    in_maps = []
    group_uids = {}  # (chunk, touched) -> unique-label ids in device order
    for ci in range(NCHUNK):
        for tf in (0, 1):
            k = ci * 2 + tf
            group_uids[(ci, tf)] = order[starts[k]:starts[k + 1]]

    def idx_input(ci, tf, pad):
        uids = group_uids[(ci, tf)]
        li = np.zeros(pad, dtype=np.int16)
        li[:uids.shape[0]] = local[uids]
        return _wrap_idxs(li)

    for core in range(NCORES):
        ca, cb = 2 * core, 2 * core + 1
        im = dict(aux)
        im["tab_a"] = tab[ca * CH:(ca + 1) * CH]
        im["tab_b"] = tab[cb * CH:(cb + 1) * CH]
        im["fwi"] = np.concatenate([
            idx_input(ca, 0, u_pad), idx_input(cb, 0, u_pad),
            idx_input(ca, 1, t_pad), idx_input(cb, 1, t_pad)], axis=1)
        in_maps.append(im)

    # column (within a core's virtual [C, totcol] output) of each unique
    # label node; _finish first unpacks the [114, totcol/2] device layout.
    totcol = 2 * (u_pad + t_pad)
    col_of_uniq = np.empty(uniq_vals.shape[0], dtype=np.int64)
    for ci in range(NCHUNK):
        core = ci // 2
        for tf in (0, 1):
            uids = group_uids[(ci, tf)]
            if (ci % 2) == 0:
                c0 = 0 if tf == 0 else u_pad
            else:
                c0 = (u_pad + t_pad) if tf == 0 else (2 * u_pad + t_pad)
            col_of_uniq[uids] = core * totcol + c0 + np.arange(uids.shape[0])

    meta = {"u_pad": u_pad, "t_pad": t_pad, "col_of_uniq": col_of_uniq,
            "inv": inv, "nb": label_nodes.shape[0]}
    return in_maps, meta


def _finish(core_outs, meta):
    """Unpack [114, tch] col-tiled outputs and map back to label order."""
    outs50 = []
    for o in core_outs:
        o = np.asarray(o, dtype=np.float32)
        nh = o.shape[1]
        arr = o.reshape(OC, nh // S, S)
        st = np.stack([arr[0:C], arr[64:64 + C]], axis=2)  # [C, nh/S, 2, S]
        outs50.append(st.reshape(C, 2 * nh))
    combined = np.concatenate(outs50, axis=1)  # [C, NCORES*totcol]
    return np.ascontiguousarray(
        combined[:, meta["col_of_uniq"][meta["inv"]]].T)


def kernel(**inputs):
    inputs = {k: np.asarray(v) for k, v in inputs.items()}
    in_maps, meta = _prepare(**inputs)
    nc = _cached_program(meta["u_pad"], meta["t_pad"], CH)
    res = run_bass_kernel_spmd(nc, in_maps, core_ids=list(range(NCORES)))
    return _finish([r["out"] for r in res.results], meta)


# revision 46
# speedup vs baseline: 1.1208x; 1.1208x over previous
"""DyRep classifier Bass kernel for 8 Trainium2 NeuronCores.

Strategy (self-contained; shapes hardcoded for the target problem):
  - The output depends only on per-label-node rows of (memory_buf,
    node_state[post-event], last_seen[post-event], node_features).
  - Host packs the four per-node tables into one 768-byte row per node
    (vs 1024B for the bf16-only layout):
      block0 (256B): memory_buf bf16 — DVE operand.
      block1 (256B): node_state bf16 — GRU matmul rhs / DVE operand.
      block2 (256B): dec_pre=exp(relu(decay)*last_seen) bf16 at u16-unit
        0, feat*4 as fp8 pairs on units 1..127 (even bytes feat[1:128],
        odd bytes feat[128:172] on units 1..44 and feat[0] on unit 45).
    dec_pre is in [1,1.65) so BOTH its bytes decode as finite fp8 — the
    feat matmuls span it with zero weight rows without NaN risk. The
    feat matmuls run fp8 x fp8 (weights also *4; HW runs mixed
    bf16 x fp8 matmuls incorrectly, and fp8 weights at natural scale
    would be subnormal). The combined x16 is undone for free by the
    ACT scale on the h1 relu; W1 is pre-multiplied by 16 to match.
    No exp on device: dec = dec_pre * exp(-rd*T) via an ACT scale on a
    [1,S] slice, broadcast across partitions by a K=1 ones-matmul.
    Sharded row-wise into 16 chunks of 31250 rows (int16-indexable);
    each of the 8 cores owns 2 chunks.
  - Host routes each unique label node to its owner (core, chunk) and
    splits into "untouched" / "touched" (touched = node hit by the
    event batch, needs the GRU update applied on the fly).
  - Device per core: dma_gather(transpose=True) delivers rows
    feature-major. h1 psum accumulates 3 matmuls (featA fp8, featB fp8,
    W1@(mem + state*dec)), ACT relu with fused bias+scale, then W2
    (zero-padded to 64 rows) col-tiled so both supertiles of a gather
    land in ONE [128, 448] psum bank (s0 -> partitions 0:64,
    s1 -> 64:128), one DVE bias-copy, bf16 half-width output.
  - Host unpermutes the per-core outputs back to label order.
"""

import functools
import os
import numpy as np
import ml_dtypes

import concourse.bass as bass
import concourse.mybir as mybir
import concourse.tile as tile
from concourse import bacc
from concourse.bass_utils import run_bass_kernel_spmd

BF16 = ml_dtypes.bfloat16
FP8 = ml_dtypes.float8_e4m3

# Problem dims (fixed by the task)
N = 500000
H = 128
F = 172
C = 50
B = 200000

NCORES = 8
NCHUNK = 16                  # index chunks (int16 addressing limit)
CH = N // NCHUNK             # 31250 rows per chunk
ROW = 512                    # bf16-unit elements per packed row (1024 bytes)
G = int(os.environ.get("K_G", "896"))   # occurrences per gather
# (transpose dma_gather num_idxs is HW-capped at 1016: 896 largest %128)
S = G // 2                   # supertile: occurrences per compute slice
FSC = 4.0                    # fp8 scale for feat AND its weights (x16 total)
OC = 114                     # packed output partitions (2 x C, col-tiled)
NSWQ = int(os.environ.get("K_NSWQ", "4"))  # SWDGE queues: alternate gathers
# across queues so one gather's ring drain overlaps the next gather's
# descriptor generation (each queue has its own 256-desc ring).

f32 = mybir.dt.float32
bf16 = mybir.dt.bfloat16
fp8 = mybir.dt.float8e4
i16 = mybir.dt.int16
AF = mybir.ActivationFunctionType
OP = mybir.AluOpType


def _wrap_idxs(idx: np.ndarray) -> np.ndarray:
    """Wrap a flat int16 index list into the [128, n/16] SWDGE layout:
    element j at [j%16, j//16], replicated into all 8 16-partition groups."""
    n = idx.shape[0]
    assert n % 16 == 0
    cols = n // 16
    t = np.empty((128, cols), dtype=np.int16)
    blk = idx.reshape(cols, 16).T  # [16, cols]
    for k in range(8):
        t[k * 16:(k + 1) * 16, :] = blk
    return t


def build_program(u_pad: int, t_pad: int, ch: int = CH):
    """Build + compile the SPMD Bass program. Cached by padded sizes."""
    nc = bacc.Bacc("TRN2", target_bir_lowering=False, debug=False,
                   num_devices=NCORES, num_swdge_queues=NSWQ)

    dt_in = {}

    def din(name, shape, dt):
        dt_in[name] = nc.dram_tensor(name, shape, dt, kind="ExternalInput").ap()
        return dt_in[name]

    tab_a = din("tab_a", (ch, ROW), bf16)
    tab_b = din("tab_b", (ch, ROW), bf16)
    # Packed weight tensors: 3 DMAs instead of ~20 serialized ones.
    # fwi: [uidx_a | uidx_b | tidx_a | tidx_b] int16
    iu, it = u_pad // 16, t_pad // 16
    fwi = din("fwi", (128, 2 * iu + 2 * it), i16)
    # fwb: [w1t | w2t | whhrt | whhzt | whhnt] bf16 + [wfab bitcast] handled
    # separately (fp8): wfab as DoubleRow planes: [k,0,:] even-byte weights
    # (feat[k], row0 zero), [k,1,:] odd-byte weights (feat[127+k] for
    # k=1..44, feat[0] at k=45, zero elsewhere); all *4
    wfab = din("wfab", (128, 2, 128), fp8)
    fwb = din("fwb", (128, 576), bf16)
    # fwf: [b1p | b2v2 | c_r | c_z | gin | bhn | dect | dtc] f32
    fwf = din("fwf", (128, 8), f32)

    tch = (u_pad + t_pad)                    # out columns per core (half)
    out = nc.dram_tensor("out", (OC, tch), bf16, kind="ExternalOutput").ap()

    class W:
        pass

    with tile.TileContext(nc) as tc:
        with tc.tile_pool(name="wpool", bufs=1) as wp:
            fwi_t = wp.tile([128, 2 * iu + 2 * it], i16, tag="fwi")
            nc.sync.dma_start(fwi_t[:], fwi[:])
            wfab_t = wp.tile([128, 2, 128], fp8, tag="wfab")
            nc.scalar.dma_start(wfab_t[:], wfab[:])
            fwb_t = wp.tile([128, 576], bf16, tag="fwb")
            nc.sync.dma_start(fwb_t[:], fwb[:])
            fwf_t = wp.tile([128, 8], f32, tag="fwf")
            nc.scalar.dma_start(fwf_t[:], fwf[:])
            W.uidx_a = fwi_t[:, 0:iu]
            W.uidx_b = fwi_t[:, iu:2 * iu]
            W.tidx_a = fwi_t[:, 2 * iu:2 * iu + it]
            W.tidx_b = fwi_t[:, 2 * iu + it:2 * iu + 2 * it]
            W.wfab = wfab_t
            W.w1t = fwb_t[:, 0:128]
            W.w2t = fwb_t[:, 128:192]
            W.whhrt = fwb_t[:, 192:320]
            W.whhzt = fwb_t[:, 320:448]
            W.whhnt = fwb_t[:, 448:576]
            W.b1p = fwf_t[:, 0:1]
            W.b2v2 = fwf_t[0:OC, 1:2]
            W.c_r = fwf_t[:, 2:3]
            W.c_z = fwf_t[:, 3:4]
            W.gin = fwf_t[:, 4:5]
            W.bhn = fwf_t[:, 5:6]
            W.dect = fwf_t[:, 6:7]
            W.dtc = fwf_t[:, 7:8]

            def fab_mm(p_h1, X8, s):
                """Open the h1 psum group: featA+featB in ONE DoubleRow
                matmul consuming the raw fp8 byte pairs of block2."""
                rhs = X8[:, 2, bass.ds(2 * s * S, 2 * S)].rearrange(
                    "p (n two) -> p two n", two=2)
                nc.tensor.matmul(p_h1[:], lhsT=W.wfab[:], rhs=rhs,
                                 start=True, stop=False,
                                 perf_mode=mybir.MatmulPerfMode.DoubleRow)

            def finish_tile(sb, ps2, p_o, X, s, p_h1, sstate):
                """Close the h1 group (W1 matmul), relu, W2 into the
                col-tiled p_o half."""
                sl = bass.ds(s * S, S)
                memT = X[:, 0, sl]
                t1 = sb.tile([128, S], bf16, tag="t1", bufs=6)
                nc.vector.tensor_tensor(out=t1[:], in0=sstate[:], in1=memT,
                                        op=OP.add)
                nc.tensor.matmul(p_h1[:], lhsT=W.w1t, rhs=t1[:],
                                 start=False, stop=True)
                h1 = sb.tile([128, S], bf16, tag="h1s", bufs=6)
                nc.scalar.activation(h1[:], p_h1[:], AF.Relu, bias=W.b1p,
                                     scale=1.0 / 16.0)
                o_sl = p_o[0:64, :] if s == 0 else p_o[64:128, :]
                nc.tensor.matmul(o_sl, lhsT=W.w2t, rhs=h1[:],
                                 start=True, stop=True)

            def untouched_gather(sb, ps2, p_o, X):
                """Both supertiles of one gather, phase-split so the
                gather-ready work (feat DoubleRow matmuls, decayed-state
                scaling off the gather-broadcast dec block) issues before
                the dependent W1/W2 chain."""
                X8 = X[:].bitcast(fp8)
                p_h1, sstates = [], []
                for s in range(2):
                    ph = ps2.tile([128, S], f32, tag="h1")
                    fab_mm(ph, X8, s)
                    p_h1.append(ph)
                    sl = bass.ds(s * S, S)
                    sstate = sb.tile([128, S], bf16, tag="sstate", bufs=6)
                    nc.vector.scalar_tensor_tensor(
                        out=sstate[:], in0=X[:, 1, sl], scalar=W.dtc,
                        in1=X[:, 3, sl], op0=OP.mult, op1=OP.mult)
                    sstates.append(sstate)
                for s in range(2):
                    finish_tile(sb, ps2, p_o, X, s, p_h1[s], sstates[s])

            def touched_supertile(sb, ps2, p_o, X, s):
                sl = bass.ds(s * S, S)
                X8 = X[:].bitcast(fp8)
                stT = X[:, 1, sl]
                if True:
                    p_r = ps2.tile([128, S], f32, tag="h1")
                    nc.tensor.matmul(p_r[:], lhsT=W.whhrt, rhs=stT,
                                     start=True, stop=True)
                    p_z = pso.tile([128, S], f32, tag="po")
                    nc.tensor.matmul(p_z[:], lhsT=W.whhzt, rhs=stT,
                                     start=True, stop=True)
                    p_n = ps2.tile([128, S], f32, tag="h1")
                    nc.tensor.matmul(p_n[:], lhsT=W.whhnt, rhs=stT,
                                     start=True, stop=True)
                    r = sb.tile([128, S], f32, tag="r")
                    nc.scalar.activation(r[:], p_r[:], AF.Sigmoid, bias=W.c_r)
                    z = sb.tile([128, S], f32, tag="z")
                    nc.scalar.activation(z[:], p_z[:], AF.Sigmoid, bias=W.c_z)
                    hn = sb.tile([128, S], f32, tag="hn")
                    nc.scalar.activation(hn[:], p_n[:], AF.Identity, bias=W.bhn)
                    rn = sb.tile([128, S], f32, tag="rn")
                    nc.vector.tensor_tensor(out=rn[:], in0=r[:], in1=hn[:],
                                            op=OP.mult)
                    n = sb.tile([128, S], f32, tag="n")
                    nc.scalar.activation(n[:], rn[:], AF.Tanh, bias=W.gin)
                    d = sb.tile([128, S], f32, tag="d")
                    nc.vector.tensor_tensor(out=d[:], in0=stT, in1=n[:],
                                            op=OP.subtract)
                    zd = sb.tile([128, S], f32, tag="zd")
                    nc.vector.tensor_tensor(out=zd[:], in0=z[:], in1=d[:],
                                            op=OP.mult)
                    ns = sb.tile([128, S], f32, tag="ns")
                    nc.vector.tensor_tensor(out=ns[:], in0=n[:], in1=zd[:],
                                            op=OP.add)
                    sstate = sb.tile([128, S], bf16, tag="sstate", bufs=6)
                    nc.vector.tensor_scalar_mul(sstate[:], ns[:],
                                                W.dect)

                p_h1 = ps2.tile([128, S], f32, tag="h1")
                fab_mm(p_h1, X8, s)
                finish_tile(sb, ps2, p_o, X, s, p_h1, sstate)

            gctr = [0]

            def stream(gp, sb, ps2, pso, table_ap, idx_tile, n_occ,
                       col0h, touched):
                """Process one (chunk, touched?) stream of n_occ occurrences
                in gathers of G; outputs to out[:, col0h : col0h+n_occ/2]."""
                n_g = n_occ // G
                for g in range(n_g):
                    X = gp.tile([128, 4, G], bf16, tag="gath")
                    nc.gpsimd.dma_gather(
                        out_ap=X[:],
                        in_ap=table_ap[:],
                        idxs_ap=idx_tile[:, bass.ds(g * G // 16, G // 16)],
                        num_idxs=G,
                        num_idxs_reg=G,
                        elem_size=ROW,
                        transpose=True,
                        queue_num=gctr[0] % NSWQ,
                    )
                    gctr[0] += 1
                    p_o = pso.tile([128, S], f32, tag="po")
                    if touched:
                        for s in range(2):
                            touched_supertile(sb, ps2, p_o, X, s)
                    else:
                        untouched_gather(sb, ps2, p_o, X)
                    osb = gp.tile([OC, S], bf16, tag="osb")
                    nc.scalar.activation(osb[:], p_o[0:OC, :], AF.Identity,
                                         bias=W.b2v2)
                    nc.sync.dma_start(
                        out[:, bass.ds(col0h + g * S, S)], osb[:])

            # One deep gather pool shared by both phases so the Pool engine
            # streams descriptor generation continuously across the
            # touched->untouched transition. Touched first: its serial GRU
            # chain drains while untouched gathers already issue.
            # Single pool scope for both phases (a pool-close between the
            # touched and untouched streams costs a ~20us all-queue drain).
            with tc.tile_pool(name="gp", bufs=8) as gp, \
                 tc.tile_pool(name="sb", bufs=2) as sb, \
                 tc.tile_pool(name="ps2", bufs=4, space="PSUM") as ps2, \
                 tc.tile_pool(name="pso", bufs=4, space="PSUM") as pso:
                stream(gp, sb, ps2, pso, tab_a, W.tidx_a, t_pad,
                       u_pad // 2, True)
                stream(gp, sb, ps2, pso, tab_b, W.tidx_b, t_pad,
                       u_pad + t_pad // 2, True)
                stream(gp, sb, ps2, pso, tab_a, W.uidx_a, u_pad,
                       0, False)
                stream(gp, sb, ps2, pso, tab_b, W.uidx_b, u_pad,
                       (u_pad + t_pad) // 2, False)

    nc.compile()
    return nc


@functools.lru_cache(maxsize=4)
def _cached_program(u_pad, t_pad, ch):
    return build_program(u_pad, t_pad, ch)


def _round_up(x, m):
    return ((x + m - 1) // m) * m


def _prepare(label_nodes, src, dst, t, msg, memory_buf, node_state, last_seen,
             node_features, decay, W_msg, b_msg, W_ih, W_hh, b_ih, b_hh,
             W_feat, b_feat, W1, b1, W2, b2, current_time):
    """Host-side routing/packing. Returns (in_maps, meta)."""
    label_nodes = np.asarray(label_nodes)

    # ---- host: event-level scalars (O(1) work) ----
    t0 = float(np.asarray(t)[0])
    T = float(current_time)
    rdecay = max(float(decay), 0.0)
    event_msg = msg[0].astype(np.float64) @ W_msg.T.astype(np.float64) + b_msg
    gi = event_msg @ W_ih.T.astype(np.float64) + b_ih  # [3H], includes b_ih
    gi = gi.astype(np.float32)
    dec_t = np.float32(np.exp(-rdecay * (T - t0)))

    # ---- host: routing (dedup to unique label nodes) ----
    touched_nodes = np.unique(np.concatenate([src, dst]))
    uniq_vals, inv = np.unique(label_nodes, return_inverse=True)
    is_t = np.isin(uniq_vals, touched_nodes)
    chunk_id = uniq_vals // CH            # 0..15
    local = (uniq_vals % CH).astype(np.int16)

    key = chunk_id.astype(np.int64) * 2 + is_t
    order = np.argsort(key, kind="stable")
    counts = np.bincount(key, minlength=NCHUNK * 2)
    u_counts = counts[0::2]
    t_counts = counts[1::2]
    u_pad = max(_round_up(int(u_counts.max()), G), G)
    t_pad = max(_round_up(int(t_counts.max()), G), G)

    starts = np.zeros(NCHUNK * 2 + 1, dtype=np.int64)
    np.cumsum(counts, out=starts[1:])

    # ---- host: packed 768B-row table ----
    tab8 = np.zeros((N, 2 * ROW), dtype=np.uint8)
    tab8[:, 0:256] = memory_buf.astype(BF16).view(np.uint8)
    tab8[:, 256:512] = node_state.astype(BF16).view(np.uint8)
    blk = np.zeros((N, 256), dtype=np.uint8)
    f8 = (node_features * FSC).astype(FP8).view(np.uint8)
    dec_pre = np.exp(rdecay * last_seen.astype(np.float64)).astype(BF16)
    blk[:, 0:2] = dec_pre.reshape(-1, 1).view(np.uint8)
    blk[:, 2:256:2] = f8[:, 1:128]        # featA: even bytes, units 1..127
    blk[:, 3:91:2] = f8[:, 128:172]       # featB: odd bytes, units 1..44
    blk[:, 91] = f8[:, 0]                 # feat[0]: odd byte, unit 45
    tab8[:, 512:768] = blk
    # block3: dec_pre replicated into all 128 units -> the transpose
    # gather broadcasts it across partitions for free.
    tab8[:, 768:1024] = np.broadcast_to(
        dec_pre.reshape(-1, 1, 1).view(np.uint8), (N, 128, 2)).reshape(N, 256)
    tab = tab8.view(BF16)                 # [N, 512] bf16-typed bytes

    # ---- host: weights / aux ----
    def bfc(x):
        return np.ascontiguousarray(x, dtype=BF16)

    def f32c(x):
        return np.ascontiguousarray(x, dtype=np.float32).reshape(-1, 1)

    WcT = (W1 @ W_feat).T * FSC  # [F, H]; W_feat folded through W1, x4
    wfab = np.zeros((128, 2, H), np.float32)
    wfab[1:128, 0] = WcT[1:128]           # even-byte plane: feat[1:128]
    wfab[1:45, 1] = WcT[128:172]          # odd-byte plane: feat[128:172]
    wfab[45, 1] = WcT[0]                  # feat[0] at odd byte of unit 45
    b2v2 = np.zeros(OC, np.float32)
    b2v2[0:C] = b2
    b2v2[64:64 + C] = b2
    fwb = np.concatenate([
        bfc(W1.T * (FSC * FSC)),
        bfc(np.concatenate(
            [W2.T, np.zeros((H, 64 - C), np.float32)], axis=1)),
        bfc(W_hh[0:128].T), bfc(W_hh[128:256].T), bfc(W_hh[256:384].T),
    ], axis=1)
    fwf = np.zeros((128, 8), np.float32)
    fwf[:, 0] = b1 + W1 @ b_feat
    fwf[0:OC, 1] = b2v2
    fwf[:, 2] = gi[0:128] + b_hh[0:128]
    fwf[:, 3] = gi[128:256] + b_hh[128:256]
    fwf[:, 4] = gi[256:384]
    fwf[:, 5] = b_hh[256:384]
    fwf[:, 6] = dec_t
    fwf[:, 7] = np.exp(-rdecay * T)
    aux = {
        "wfab": np.ascontiguousarray(wfab, dtype=FP8),
        "fwb": np.ascontiguousarray(fwb),
        "fwf": fwf,
    }

    # ---- host: per-core input maps ----
    in_maps = []
    group_uids = {}  # (chunk, touched) -> unique-label ids in device order
    for ci in range(NCHUNK):
        for tf in (0, 1):
            k = ci * 2 + tf
            group_uids[(ci, tf)] = order[starts[k]:starts[k + 1]]

    def idx_input(ci, tf, pad):
        uids = group_uids[(ci, tf)]
        li = np.zeros(pad, dtype=np.int16)
        li[:uids.shape[0]] = local[uids]
        return _wrap_idxs(li)

    for core in range(NCORES):
        ca, cb = 2 * core, 2 * core + 1
        im = dict(aux)
        im["tab_a"] = tab[ca * CH:(ca + 1) * CH]
        im["tab_b"] = tab[cb * CH:(cb + 1) * CH]
        im["fwi"] = np.concatenate([
            idx_input(ca, 0, u_pad), idx_input(cb, 0, u_pad),
            idx_input(ca, 1, t_pad), idx_input(cb, 1, t_pad)], axis=1)
        in_maps.append(im)

    # column (within a core's virtual [C, totcol] output) of each unique
    # label node; _finish first unpacks the [114, totcol/2] device layout.
    totcol = 2 * (u_pad + t_pad)
    col_of_uniq = np.empty(uniq_vals.shape[0], dtype=np.int64)
    for ci in range(NCHUNK):
        core = ci // 2
        for tf in (0, 1):
            uids = group_uids[(ci, tf)]
            if (ci % 2) == 0:
                c0 = 0 if tf == 0 else u_pad
            else:
                c0 = (u_pad + t_pad) if tf == 0 else (2 * u_pad + t_pad)
            col_of_uniq[uids] = core * totcol + c0 + np.arange(uids.shape[0])

    meta = {"u_pad": u_pad, "t_pad": t_pad, "col_of_uniq": col_of_uniq,
            "inv": inv, "nb": label_nodes.shape[0]}
    return in_maps, meta


def _finish(core_outs, meta):
    """Unpack [114, tch] col-tiled outputs and map back to label order."""
    outs50 = []
    for o in core_outs:
        o = np.asarray(o, dtype=np.float32)
        nh = o.shape[1]
        arr = o.reshape(OC, nh // S, S)
        st = np.stack([arr[0:C], arr[64:64 + C]], axis=2)  # [C, nh/S, 2, S]
        outs50.append(st.reshape(C, 2 * nh))
    combined = np.concatenate(outs50, axis=1)  # [C, NCORES*totcol]
    return np.ascontiguousarray(
        combined[:, meta["col_of_uniq"][meta["inv"]]].T)


def kernel(**inputs):
    inputs = {k: np.asarray(v) for k, v in inputs.items()}
    in_maps, meta = _prepare(**inputs)
    nc = _cached_program(meta["u_pad"], meta["t_pad"], CH)
    res = run_bass_kernel_spmd(nc, in_maps, core_ids=list(range(NCORES)))
    return _finish([r["out"] for r in res.results], meta)
